# revision 11
# baseline (speedup 1.0000x reference)
"""LLaMA attention block (b=1, s=2048, d=2048, 16 heads) on 8 TRN2 NeuronCores.

Sharding: tensor-parallel over heads (2 heads per core). Each core computes
q/k/v projections for its head slice, RoPE, full (non-causal) attention for its
heads, and a partial output projection; the host sums the 8 partial outputs.

Device-side layout notes (per core):
  - x is passed transposed (xT, d-major) so projections contract over the
    partition dim without on-device transposes.
  - q/k are produced transposed per head: qT/kT [dh=128, s].
  - scores are computed transposed: scoresT [k, q] so exp evicts PSUM->SBUF
    and PV consumes probsT directly (lhsT = v in natural [s, dh] layout).
  - softmax has no max-subtraction (inputs are unit-scale gaussians; scores
    std ~1 after 1/sqrt(dh), exp cannot overflow fp32).
  - row-sums of probsT via ones-column matmul; 1/sum via DVE approx
    reciprocal; broadcast via K=1 ones-row matmul.
  - all matmuls run in fp32r (full PE rate at N>=256, ~1e-4 relative error).
"""
import numpy as np
from contextlib import ExitStack

S, D, NH, DH = 2048, 2048, 16, 128
NCORES = 8
HPC = NH // NCORES          # heads per core
DHC = HPC * DH              # per-core projection width (256)
ROPE_BASE = 10000.0

_CACHE = {}


def _build(s, d):
    import concourse.bacc as bacc
    import concourse.mybir as mybir
    import concourse.tile as tile

    F32 = mybir.dt.float32
    F32R = mybir.dt.float32r
    AF = mybir.ActivationFunctionType

    KB = d // 128          # contraction chunks for projections
    NS = s // 512          # s-slices for projections / y columns
    MB = s // 128          # s-blocks for output rows
    QS = s // 512          # q-slices for attention
    SCALE = 1.0 / float(np.sqrt(DH))

    nc = bacc.Bacc("TRN2", target_bir_lowering=False, debug=False)

    xT_d = nc.dram_tensor("xT", [KB, 128, s], F32, kind="ExternalInput")
    wq_d = nc.dram_tensor("wqT", [KB, 128, DHC], F32, kind="ExternalInput")
    wk_d = nc.dram_tensor("wkT", [KB, 128, DHC], F32, kind="ExternalInput")
    wv_d = nc.dram_tensor("wvT", [KB, 128, DHC], F32, kind="ExternalInput")
    wo_d = nc.dram_tensor("woT", [HPC, 128, s], F32, kind="ExternalInput")
    cos_d = nc.dram_tensor("cosT", [128, s], F32, kind="ExternalInput")
    ssin_d = nc.dram_tensor("ssinT", [128, s], F32, kind="ExternalInput")
    onescol_d = nc.dram_tensor("ones_col", [128, 1], F32, kind="ExternalInput")
    onesrow_d = nc.dram_tensor("ones_row", [1, 128], F32, kind="ExternalInput")
    perm_d = nc.dram_tensor("perm64", [128, 128], F32, kind="ExternalInput")
    y_d = nc.dram_tensor("y", [MB, 128, s], F32, kind="ExternalOutput")

    with tile.TileContext(nc) as tc:
        with ExitStack() as root:
            consts = root.enter_context(tc.tile_pool(name="consts", bufs=1))
            ones_col = consts.tile([128, 1], F32R, name="ones_col_s")
            nc.sync.dma_start(out=ones_col[:], in_=onescol_d[:].bitcast(F32R))
            ones_rowF = consts.tile([128, 128], F32R, name="ones_row_s")
            nc.sync.dma_start(out=ones_rowF[0:1, :], in_=onesrow_d[:].bitcast(F32R))
            ones_row = ones_rowF[0:1, :]
            perm_s = consts.tile([128, 128], F32R, name="perm_s")
            nc.sync.dma_start(out=perm_s[:], in_=perm_d[:].bitcast(F32R))
            cos_s = consts.tile([128, s], F32, name="cos_s")
            nc.sync.dma_start(out=cos_s[:], in_=cos_d[:])
            ssin_s = consts.tile([128, s], F32, name="ssin_s")
            nc.sync.dma_start(out=ssin_s[:], in_=ssin_d[:])

            wo_pool = root.enter_context(tc.tile_pool(name="wo_pool", bufs=1))
            wo_s = [wo_pool.tile([128, s], F32R, name=f"wo{h}") for h in range(HPC)]
            for h in range(HPC):
                nc.sync.dma_start(out=wo_s[h][:], in_=wo_d[h].bitcast(F32R))

            v_pool = root.enter_context(tc.tile_pool(name="v_pool", bufs=1))
            v_s = [v_pool.tile([128, DHC], F32R, name=f"v{i}") for i in range(MB)]

            rot_pool = root.enter_context(tc.tile_pool(name="rot_pool", bufs=1))
            qrot = [rot_pool.tile([128, s], F32R, name=f"qrot{m}") for m in range(HPC)]
            krot = [rot_pool.tile([128, s], F32R, name=f"krot{m}") for m in range(HPC)]

            # ---------- phase 1+2: q/k/v projections fused with RoPE ----------
            ph1 = ExitStack()
            wqkv = ph1.enter_context(tc.tile_pool(name="wqkv", bufs=1))
            wq_s = [wqkv.tile([128, DHC], F32R, name=f"wq{i}") for i in range(KB)]
            wk_s = [wqkv.tile([128, DHC], F32R, name=f"wk{i}") for i in range(KB)]
            wv_s = [wqkv.tile([128, DHC], F32R, name=f"wv{i}") for i in range(KB)]
            for i in range(KB):
                nc.sync.dma_start(out=wq_s[i][:], in_=wq_d[i].bitcast(F32R))
                nc.sync.dma_start(out=wk_s[i][:], in_=wk_d[i].bitcast(F32R))
                nc.sync.dma_start(out=wv_s[i][:], in_=wv_d[i].bitcast(F32R))

            qkpre = ph1.enter_context(tc.tile_pool(name="qkpre", bufs=1))
            qT_s = [qkpre.tile([128, s], F32R, name=f"qT{m}") for m in range(HPC)]
            kT_s = [qkpre.tile([128, s], F32R, name=f"kT{m}") for m in range(HPC)]

            xk_pool = ph1.enter_context(tc.tile_pool(name="xk_pool", bufs=4))
            qk_ps = ph1.enter_context(tc.tile_pool(name="qk_ps", bufs=1, space="PSUM"))
            v_ps = ph1.enter_context(tc.tile_pool(name="v_ps", bufs=1, space="PSUM"))

            # (pre-rope source, rotated dest) streams, head-0 first
            streams = [(qT_s[0], qrot[0]), (kT_s[0], krot[0]),
                       (qT_s[1], qrot[1]), (kT_s[1], krot[1])]

            for n in range(NS):
                ns = slice(512 * n, 512 * (n + 1))
                pq = [qk_ps.tile([128, 512], F32, name=f"pq{n}_{m}", tag=f"pq{m}")
                      for m in range(HPC)]
                pk = [qk_ps.tile([128, 512], F32, name=f"pk{n}_{m}", tag=f"pk{m}")
                      for m in range(HPC)]
                pv = [v_ps.tile([128, DHC], F32, name=f"pv{n}_{j}", tag=f"pv{j}")
                      for j in range(4)]
                for kb in range(KB):
                    xk = xk_pool.tile([128, 512], F32R, name=f"xk{n}_{kb}", tag="xk")
                    nc.sync.dma_start(out=xk[:], in_=xT_d[kb][:, ns].bitcast(F32R))
                    st = kb == 0
                    sp = kb == KB - 1
                    for m in range(HPC):
                        ms = slice(128 * m, 128 * (m + 1))
                        nc.tensor.matmul(pq[m][:], wq_s[kb][:, ms], xk[:], start=st, stop=sp)
                        nc.tensor.matmul(pk[m][:], wk_s[kb][:, ms], xk[:], start=st, stop=sp)
                    for j in range(4):
                        js = slice(128 * j, 128 * (j + 1))
                        nc.tensor.matmul(pv[j][:], xk[:, js], wv_s[kb][:],
                                         start=st, stop=sp)
                for m in range(HPC):
                    nc.scalar.copy(qT_s[m][:, ns], pq[m][:])
                    nc.scalar.copy(kT_s[m][:, ns], pk[m][:])
                for j in range(4):
                    nc.scalar.copy(v_s[4 * n + j][:], pv[j][:])

            ph1.close()

            # ---------------- phase 2: RoPE (per 512-slice) ----------------
            ph2 = ExitStack()
            rope_ps = ph2.enter_context(tc.tile_pool(name="rope_ps", bufs=2, space="PSUM"))
            t1_pool = ph2.enter_context(tc.tile_pool(name="t1_pool", bufs=2))
            t2_pool = ph2.enter_context(tc.tile_pool(name="t2_pool", bufs=2))
            for ri, (rsrc, dst) in enumerate(streams):
                for n in range(NS):
                    ns = slice(512 * n, 512 * (n + 1))
                    shift = rope_ps.tile([128, 512], F32, name=f"sh{ri}_{n}", tag="shift")
                    nc.tensor.matmul(shift[:], perm_s[:], rsrc[:, ns],
                                     start=True, stop=True)
                    t1 = t1_pool.tile([128, 512], F32, name=f"t1_{ri}_{n}", tag="t1")
                    nc.gpsimd.tensor_mul(t1[:], rsrc[:, ns].bitcast(F32), cos_s[:, ns])
                    t2 = t2_pool.tile([128, 512], F32, name=f"t2_{ri}_{n}", tag="t2")
                    nc.vector.tensor_mul(t2[:], shift[:], ssin_s[:, ns])
                    nc.vector.tensor_add(dst[:, ns], t1[:], t2[:])
            ph2.close()

            # ---------------- phase 3: attention per head ----------------
            oT_pool = root.enter_context(tc.tile_pool(name="oT_pool", bufs=1))
            oT_s = [oT_pool.tile([128, s], F32R, name=f"oT{h}") for h in range(HPC)]

            ph3 = ExitStack()
            sc_ps = ph3.enter_context(tc.tile_pool(name="sc_ps", bufs=2, space="PSUM"))
            o_ps = ph3.enter_context(tc.tile_pool(name="o_ps", bufs=1, space="PSUM"))
            rmisc_ps = ph3.enter_context(tc.tile_pool(name="rmisc_ps", bufs=1, space="PSUM"))
            probs_pool = ph3.enter_context(tc.tile_pool(name="probs_pool", bufs=3))
            rinv_pool = ph3.enter_context(tc.tile_pool(name="rinv_pool", bufs=2))
            rb_pool = ph3.enter_context(tc.tile_pool(name="rb_pool", bufs=2))

            QW = min(1024, s)
            for h in range(HPC):
                hs = slice(128 * h, 128 * (h + 1))
                for q in range(s // QW):
                    qs = slice(QW * q, QW * (q + 1))
                    qs_a = slice(QW * q, QW * q + 512)
                    qs_b = slice(QW * q + 512, QW * (q + 1))
                    osum = o_ps.tile([128, QW], F32, name=f"osum{h}_{q}", tag="osum")
                    rsum = rmisc_ps.tile([1, QW], F32, name=f"rsum{h}_{q}", tag="rmisc")
                    for kb in range(MB):
                        ks = slice(128 * kb, 128 * (kb + 1))
                        sc = sc_ps.tile([128, QW], F32, name=f"sc{h}_{q}_{kb}", tag="sc")
                        nc.tensor.matmul(sc[:, 0:512], krot[h][:, ks], qrot[h][:, qs_a],
                                         start=True, stop=True)
                        if QW > 512:
                            nc.tensor.matmul(sc[:, 512:QW], krot[h][:, ks], qrot[h][:, qs_b],
                                             start=True, stop=True)
                        pr = probs_pool.tile([128, QW], F32R,
                                             name=f"pr{h}_{q}_{kb}", tag="pr")
                        nc.scalar.activation(pr[:], sc[:], AF.Exp, scale=SCALE)
                        st = kb == 0
                        sp = kb == MB - 1
                        nc.tensor.matmul(osum[:, 0:512], v_s[kb][:, hs], pr[:, 0:512],
                                         start=st, stop=sp)
                        nc.tensor.matmul(rsum[:, 0:512], ones_col[:], pr[:, 0:512],
                                         start=st, stop=sp)
                        if QW > 512:
                            nc.tensor.matmul(osum[:, 512:QW], v_s[kb][:, hs], pr[:, 512:QW],
                                             start=st, stop=sp)
                            nc.tensor.matmul(rsum[:, 512:QW], ones_col[:], pr[:, 512:QW],
                                             start=st, stop=sp)
                    rsumsF = rinv_pool.tile([128, QW], F32, name=f"rsums{h}_{q}", tag="rsums")
                    rsums = rsumsF[0:1, :]
                    nc.scalar.copy(rsums, rsum[:])
                    rinvfF = rinv_pool.tile([128, QW], F32, name=f"rinvf{h}_{q}", tag="rinvf")
                    rinvf = rinvfF[0:1, :]
                    nc.vector.reciprocal_approx_fast(rinvf, rsums)
                    rinvrF = rinv_pool.tile([128, QW], F32R, name=f"rinvr{h}_{q}", tag="rinvr")
                    rinvr = rinvrF[0:1, :]
                    nc.scalar.copy(rinvr, rinvf)
                    rb = sc_ps.tile([128, QW], F32, name=f"rbp{h}_{q}", tag="sc")
                    nc.tensor.matmul(rb[:, 0:512], ones_row, rinvr[:, 0:512],
                                     start=True, stop=True)
                    if QW > 512:
                        nc.tensor.matmul(rb[:, 512:QW], ones_row, rinvr[:, 512:QW],
                                         start=True, stop=True)
                    rbs = rb_pool.tile([128, QW], F32, name=f"rbs{h}_{q}", tag="rbs")
                    nc.scalar.copy(rbs[:], rb[:])
                    nc.vector.tensor_mul(oT_s[h][:, qs], osum[:], rbs[:])

            ph3.close()

            # ---------------- phase 4: output projection (partial) ----------------
            ph4 = ExitStack()
            y_ps = ph4.enter_context(tc.tile_pool(name="y_ps", bufs=2, space="PSUM"))
            y_sb = ph4.enter_context(tc.tile_pool(name="y_sb", bufs=3))
            for mb in range(MB):
                ms = slice(128 * mb, 128 * (mb + 1))
                yp = y_ps.tile([128, s], F32, name=f"yp{mb}", tag="yp")
                for h in range(HPC):
                    for n in range(NS):
                        ns = slice(512 * n, 512 * (n + 1))
                        nc.tensor.matmul(yp[:, ns], oT_s[h][:, ms], wo_s[h][:, ns],
                                         start=(h == 0), stop=(h == HPC - 1))
                ys = y_sb.tile([128, s], F32, name=f"ys{mb}", tag="ys")
                half = s // 2
                nc.scalar.copy(ys[:, 0:half], yp[:, 0:half])
                nc.vector.tensor_copy(ys[:, half:s], yp[:, half:s])
                nc.sync.dma_start(out=y_d[mb], in_=ys[:])
            ph4.close()

    nc.compile()
    return nc


def _prepare_inputs(hidden_states, wq, wk, wv, wo, position_ids, s, d):
    """Host-side sharding/layout prep. Returns per-core input maps."""
    x = np.asarray(hidden_states, np.float32).reshape(s, d)
    kb = d // 128
    xT = np.ascontiguousarray(x.T).reshape(kb, 128, s)

    pos = np.asarray(position_ids).reshape(-1)[:s].astype(np.float64)
    inv_freq = 1.0 / (ROPE_BASE ** (np.arange(0, DH, 2, dtype=np.float64) / DH))
    freqs = np.outer(pos, inv_freq)                      # [s, dh/2]
    emb = np.concatenate([freqs, freqs], axis=-1)        # [s, dh]
    cosT = np.ascontiguousarray(np.cos(emb).T.astype(np.float32))   # [dh, s]
    sinT = np.ascontiguousarray(np.sin(emb).T.astype(np.float32))
    ssinT = sinT.copy()
    ssinT[: DH // 2] *= -1.0

    perm64 = np.zeros((128, 128), np.float32)
    for m in range(128):
        perm64[(m + 64) % 128, m] = 1.0

    wq = np.asarray(wq, np.float32)
    wk = np.asarray(wk, np.float32)
    wv = np.asarray(wv, np.float32)
    wo = np.asarray(wo, np.float32)

    in_maps = []
    for c in range(NCORES):
        cs = slice(DHC * c, DHC * (c + 1))
        wqT = np.ascontiguousarray(wq[cs, :].T).reshape(kb, 128, DHC)
        wkT = np.ascontiguousarray(wk[cs, :].T).reshape(kb, 128, DHC)
        wvT = np.ascontiguousarray(wv[cs, :].T).reshape(kb, 128, DHC)
        woT = np.ascontiguousarray(wo[:, cs].T).reshape(HPC, 128, d)
        in_maps.append(dict(
            xT=xT, wqT=wqT, wkT=wkT, wvT=wvT, woT=woT,
            cosT=cosT, ssinT=ssinT,
            ones_col=np.ones((128, 1), np.float32),
            ones_row=np.ones((1, 128), np.float32),
            perm64=perm64,
        ))
    return in_maps


def kernel(hidden_states, wq, wk, wv, wo, position_ids):
    from concourse.bass_utils import run_bass_kernel_spmd

    b, s, d = hidden_states.shape
    if "nc" not in _CACHE:
        _CACHE["nc"] = _build(s, d)
    nc = _CACHE["nc"]

    in_maps = _prepare_inputs(hidden_states, wq, wk, wv, wo, position_ids, s, d)
    res = run_bass_kernel_spmd(nc, in_maps, core_ids=list(range(NCORES)))
    y = np.zeros((s, d), np.float64)
    for c in range(NCORES):
        y += res.results[c]["y"].reshape(s, d).astype(np.float64)
    return y.astype(np.float32).reshape(b, s, d)


# revision 12
# speedup vs baseline: 1.0128x; 1.0128x over previous
"""LLaMA attention block (b=1, s=2048, d=2048, 16 heads) on 8 TRN2 NeuronCores.

Sharding: tensor-parallel over heads (2 heads per core). Each core computes
q/k/v projections for its head slice, RoPE, full (non-causal) attention for its
heads, and a partial output projection; the host sums the 8 partial outputs.

Device-side layout notes (per core):
  - x is passed transposed (xT, d-major) so projections contract over the
    partition dim without on-device transposes.
  - q/k are produced transposed per head: qT/kT [dh=128, s].
  - scores are computed transposed: scoresT [k, q] so exp evicts PSUM->SBUF
    and PV consumes probsT directly (lhsT = v in natural [s, dh] layout).
  - softmax has no max-subtraction (inputs are unit-scale gaussians; scores
    std ~1 after 1/sqrt(dh), exp cannot overflow fp32).
  - row-sums of probsT via ones-column matmul; 1/sum via DVE approx
    reciprocal; broadcast via K=1 ones-row matmul.
  - all matmuls run in fp32r (full PE rate at N>=256, ~1e-4 relative error).
"""
import numpy as np
from contextlib import ExitStack

S, D, NH, DH = 2048, 2048, 16, 128
NCORES = 8
HPC = NH // NCORES          # heads per core
DHC = HPC * DH              # per-core projection width (256)
ROPE_BASE = 10000.0

_CACHE = {}


def _build(s, d):
    import concourse.bacc as bacc
    import concourse.mybir as mybir
    import concourse.tile as tile

    F32 = mybir.dt.float32
    F32R = mybir.dt.float32r
    AF = mybir.ActivationFunctionType

    KB = d // 128          # contraction chunks for projections
    NS = s // 512          # s-slices for projections / y columns
    MB = s // 128          # s-blocks for output rows
    QS = s // 512          # q-slices for attention
    SCALE = 1.0 / float(np.sqrt(DH))

    nc = bacc.Bacc("TRN2", target_bir_lowering=False, debug=False)

    xT_d = nc.dram_tensor("xT", [KB, 128, s], F32, kind="ExternalInput")
    wq_d = nc.dram_tensor("wqT", [KB, 128, DHC], F32, kind="ExternalInput")
    wk_d = nc.dram_tensor("wkT", [KB, 128, DHC], F32, kind="ExternalInput")
    wv_d = nc.dram_tensor("wvT", [KB, 128, DHC], F32, kind="ExternalInput")
    wo_d = nc.dram_tensor("woT", [HPC, 128, s], F32, kind="ExternalInput")
    cos_d = nc.dram_tensor("cosT", [128, s], F32, kind="ExternalInput")
    ssin_d = nc.dram_tensor("ssinT", [128, s], F32, kind="ExternalInput")
    onescol_d = nc.dram_tensor("ones_col", [128, 1], F32, kind="ExternalInput")
    onesrow_d = nc.dram_tensor("ones_row", [1, 128], F32, kind="ExternalInput")
    perm_d = nc.dram_tensor("perm64", [128, 128], F32, kind="ExternalInput")
    y_d = nc.dram_tensor("y", [MB, 128, s], F32, kind="ExternalOutput")

    with tile.TileContext(nc) as tc:
        with ExitStack() as root:
            consts = root.enter_context(tc.tile_pool(name="consts", bufs=1))
            ones_col = consts.tile([128, 1], F32R, name="ones_col_s")
            nc.sync.dma_start(out=ones_col[:], in_=onescol_d[:].bitcast(F32R))
            ones_rowF = consts.tile([128, 128], F32R, name="ones_row_s")
            nc.sync.dma_start(out=ones_rowF[0:1, :], in_=onesrow_d[:].bitcast(F32R))
            ones_row = ones_rowF[0:1, :]
            perm_s = consts.tile([128, 128], F32R, name="perm_s")
            nc.sync.dma_start(out=perm_s[:], in_=perm_d[:].bitcast(F32R))
            cos_s = consts.tile([128, s], F32, name="cos_s")
            nc.sync.dma_start(out=cos_s[:], in_=cos_d[:])
            ssin_s = consts.tile([128, s], F32, name="ssin_s")
            nc.sync.dma_start(out=ssin_s[:], in_=ssin_d[:])

            wo_pool = root.enter_context(tc.tile_pool(name="wo_pool", bufs=1))
            wo_s = [wo_pool.tile([128, s], F32R, name=f"wo{h}") for h in range(HPC)]
            for h in range(HPC):
                nc.sync.dma_start(out=wo_s[h][:], in_=wo_d[h].bitcast(F32R))

            v_pool = root.enter_context(tc.tile_pool(name="v_pool", bufs=1))
            v_s = [v_pool.tile([128, DHC], F32R, name=f"v{i}") for i in range(MB)]

            rot_pool = root.enter_context(tc.tile_pool(name="rot_pool", bufs=1))
            qrot = [rot_pool.tile([128, s], F32R, name=f"qrot{m}") for m in range(HPC)]
            krot = [rot_pool.tile([128, s], F32R, name=f"krot{m}") for m in range(HPC)]

            # ---------- phase 1+2: q/k/v projections fused with RoPE ----------
            ph1 = ExitStack()
            wqkv = ph1.enter_context(tc.tile_pool(name="wqkv", bufs=1))
            wq_s = [wqkv.tile([128, DHC], F32R, name=f"wq{i}") for i in range(KB)]
            wk_s = [wqkv.tile([128, DHC], F32R, name=f"wk{i}") for i in range(KB)]
            wv_s = [wqkv.tile([128, DHC], F32R, name=f"wv{i}") for i in range(KB)]
            for i in range(KB):
                nc.sync.dma_start(out=wq_s[i][:], in_=wq_d[i].bitcast(F32R))
                nc.sync.dma_start(out=wk_s[i][:], in_=wk_d[i].bitcast(F32R))
                nc.sync.dma_start(out=wv_s[i][:], in_=wv_d[i].bitcast(F32R))

            qkpre = ph1.enter_context(tc.tile_pool(name="qkpre", bufs=1))
            qT_s = [qkpre.tile([128, s], F32R, name=f"qT{m}") for m in range(HPC)]
            kT_s = [qkpre.tile([128, s], F32R, name=f"kT{m}") for m in range(HPC)]

            xk_pool = ph1.enter_context(tc.tile_pool(name="xk_pool", bufs=8))
            qk_ps = ph1.enter_context(tc.tile_pool(name="qk_ps", bufs=1, space="PSUM"))
            v_ps = ph1.enter_context(tc.tile_pool(name="v_ps", bufs=1, space="PSUM"))

            # (pre-rope source, rotated dest) streams, head-0 first
            streams = [(qT_s[0], qrot[0]), (kT_s[0], krot[0]),
                       (qT_s[1], qrot[1]), (kT_s[1], krot[1])]

            for n in range(NS):
                ns = slice(512 * n, 512 * (n + 1))
                pq = [qk_ps.tile([128, 512], F32, name=f"pq{n}_{m}", tag=f"pq{m}")
                      for m in range(HPC)]
                pk = [qk_ps.tile([128, 512], F32, name=f"pk{n}_{m}", tag=f"pk{m}")
                      for m in range(HPC)]
                pv = [v_ps.tile([128, DHC], F32, name=f"pv{n}_{j}", tag=f"pv{j}")
                      for j in range(4)]
                for kb in range(KB):
                    xk = xk_pool.tile([128, 512], F32R, name=f"xk{n}_{kb}", tag="xk")
                    dmaeng = nc.sync if kb % 2 == 0 else nc.scalar
                    dmaeng.dma_start(out=xk[:], in_=xT_d[kb][:, ns].bitcast(F32R))
                    st = kb == 0
                    sp = kb == KB - 1
                    for m in range(HPC):
                        ms = slice(128 * m, 128 * (m + 1))
                        nc.tensor.matmul(pq[m][:], wq_s[kb][:, ms], xk[:], start=st, stop=sp)
                        nc.tensor.matmul(pk[m][:], wk_s[kb][:, ms], xk[:], start=st, stop=sp)
                    for j in range(4):
                        js = slice(128 * j, 128 * (j + 1))
                        nc.tensor.matmul(pv[j][:], xk[:, js], wv_s[kb][:],
                                         start=st, stop=sp)
                for m in range(HPC):
                    nc.scalar.copy(qT_s[m][:, ns], pq[m][:])
                    nc.scalar.copy(kT_s[m][:, ns], pk[m][:])
                for j in range(4):
                    nc.scalar.copy(v_s[4 * n + j][:], pv[j][:])

            ph1.close()

            # ---------------- phase 2: RoPE (per 512-slice) ----------------
            ph2 = ExitStack()
            rope_ps = ph2.enter_context(tc.tile_pool(name="rope_ps", bufs=2, space="PSUM"))
            t1_pool = ph2.enter_context(tc.tile_pool(name="t1_pool", bufs=2))
            t2_pool = ph2.enter_context(tc.tile_pool(name="t2_pool", bufs=2))
            for ri, (rsrc, dst) in enumerate(streams):
                for n in range(NS):
                    ns = slice(512 * n, 512 * (n + 1))
                    shift = rope_ps.tile([128, 512], F32, name=f"sh{ri}_{n}", tag="shift")
                    nc.tensor.matmul(shift[:], perm_s[:], rsrc[:, ns],
                                     start=True, stop=True)
                    t1 = t1_pool.tile([128, 512], F32, name=f"t1_{ri}_{n}", tag="t1")
                    nc.gpsimd.tensor_mul(t1[:], rsrc[:, ns].bitcast(F32), cos_s[:, ns])
                    t2 = t2_pool.tile([128, 512], F32, name=f"t2_{ri}_{n}", tag="t2")
                    nc.vector.tensor_mul(t2[:], shift[:], ssin_s[:, ns])
                    nc.vector.tensor_add(dst[:, ns], t1[:], t2[:])
            ph2.close()

            # ---------------- phase 3: attention per head ----------------
            oT_pool = root.enter_context(tc.tile_pool(name="oT_pool", bufs=1))
            oT_s = [oT_pool.tile([128, s], F32R, name=f"oT{h}") for h in range(HPC)]

            ph3 = ExitStack()
            sc_ps = ph3.enter_context(tc.tile_pool(name="sc_ps", bufs=2, space="PSUM"))
            o_ps = ph3.enter_context(tc.tile_pool(name="o_ps", bufs=1, space="PSUM"))
            rmisc_ps = ph3.enter_context(tc.tile_pool(name="rmisc_ps", bufs=1, space="PSUM"))
            probs_pool = ph3.enter_context(tc.tile_pool(name="probs_pool", bufs=4))
            rinv_pool = ph3.enter_context(tc.tile_pool(name="rinv_pool", bufs=2))
            rb_pool = ph3.enter_context(tc.tile_pool(name="rb_pool", bufs=2))

            QW = min(1024, s)
            for h in range(HPC):
                hs = slice(128 * h, 128 * (h + 1))
                for q in range(s // QW):
                    qs = slice(QW * q, QW * (q + 1))
                    qs_a = slice(QW * q, QW * q + 512)
                    qs_b = slice(QW * q + 512, QW * (q + 1))
                    osum = o_ps.tile([128, QW], F32, name=f"osum{h}_{q}", tag="osum")
                    rsum = rmisc_ps.tile([1, QW], F32, name=f"rsum{h}_{q}", tag="rmisc")
                    for kb in range(MB):
                        ks = slice(128 * kb, 128 * (kb + 1))
                        sc = sc_ps.tile([128, QW], F32, name=f"sc{h}_{q}_{kb}", tag="sc")
                        nc.tensor.matmul(sc[:, 0:512], krot[h][:, ks], qrot[h][:, qs_a],
                                         start=True, stop=True)
                        if QW > 512:
                            nc.tensor.matmul(sc[:, 512:QW], krot[h][:, ks], qrot[h][:, qs_b],
                                             start=True, stop=True)
                        pr = probs_pool.tile([128, QW], F32R,
                                             name=f"pr{h}_{q}_{kb}", tag="pr")
                        nc.scalar.activation(pr[:], sc[:], AF.Exp, scale=SCALE)
                        st = kb == 0
                        sp = kb == MB - 1
                        nc.tensor.matmul(osum[:, 0:512], v_s[kb][:, hs], pr[:, 0:512],
                                         start=st, stop=sp)
                        nc.tensor.matmul(rsum[:, 0:512], ones_col[:], pr[:, 0:512],
                                         start=st, stop=sp)
                        if QW > 512:
                            nc.tensor.matmul(osum[:, 512:QW], v_s[kb][:, hs], pr[:, 512:QW],
                                             start=st, stop=sp)
                            nc.tensor.matmul(rsum[:, 512:QW], ones_col[:], pr[:, 512:QW],
                                             start=st, stop=sp)
                    rsumsF = rinv_pool.tile([128, QW], F32, name=f"rsums{h}_{q}", tag="rsums")
                    rsums = rsumsF[0:1, :]
                    nc.scalar.copy(rsums, rsum[:])
                    rinvfF = rinv_pool.tile([128, QW], F32, name=f"rinvf{h}_{q}", tag="rinvf")
                    rinvf = rinvfF[0:1, :]
                    nc.vector.reciprocal_approx_fast(rinvf, rsums)
                    rinvrF = rinv_pool.tile([128, QW], F32R, name=f"rinvr{h}_{q}", tag="rinvr")
                    rinvr = rinvrF[0:1, :]
                    nc.scalar.copy(rinvr, rinvf)
                    rb = sc_ps.tile([128, QW], F32, name=f"rbp{h}_{q}", tag="sc")
                    nc.tensor.matmul(rb[:, 0:512], ones_row, rinvr[:, 0:512],
                                     start=True, stop=True)
                    if QW > 512:
                        nc.tensor.matmul(rb[:, 512:QW], ones_row, rinvr[:, 512:QW],
                                         start=True, stop=True)
                    rbs = rb_pool.tile([128, QW], F32, name=f"rbs{h}_{q}", tag="rbs")
                    nc.scalar.copy(rbs[:], rb[:])
                    nc.vector.tensor_mul(oT_s[h][:, qs], osum[:], rbs[:])

            ph3.close()

            # ---------------- phase 4: output projection (partial) ----------------
            ph4 = ExitStack()
            y_ps = ph4.enter_context(tc.tile_pool(name="y_ps", bufs=2, space="PSUM"))
            y_sb = ph4.enter_context(tc.tile_pool(name="y_sb", bufs=3))
            for mb in range(MB):
                ms = slice(128 * mb, 128 * (mb + 1))
                yp = y_ps.tile([128, s], F32, name=f"yp{mb}", tag="yp")
                for h in range(HPC):
                    for n in range(NS):
                        ns = slice(512 * n, 512 * (n + 1))
                        nc.tensor.matmul(yp[:, ns], oT_s[h][:, ms], wo_s[h][:, ns],
                                         start=(h == 0), stop=(h == HPC - 1))
                ys = y_sb.tile([128, s], F32, name=f"ys{mb}", tag="ys")
                half = s // 2
                nc.scalar.copy(ys[:, 0:half], yp[:, 0:half])
                nc.vector.tensor_copy(ys[:, half:s], yp[:, half:s])
                nc.scalar.dma_start(out=y_d[mb], in_=ys[:])
            ph4.close()

    nc.compile()
    return nc


def _prepare_inputs(hidden_states, wq, wk, wv, wo, position_ids, s, d):
    """Host-side sharding/layout prep. Returns per-core input maps."""
    x = np.asarray(hidden_states, np.float32).reshape(s, d)
    kb = d // 128
    xT = np.ascontiguousarray(x.T).reshape(kb, 128, s)

    pos = np.asarray(position_ids).reshape(-1)[:s].astype(np.float64)
    inv_freq = 1.0 / (ROPE_BASE ** (np.arange(0, DH, 2, dtype=np.float64) / DH))
    freqs = np.outer(pos, inv_freq)                      # [s, dh/2]
    emb = np.concatenate([freqs, freqs], axis=-1)        # [s, dh]
    cosT = np.ascontiguousarray(np.cos(emb).T.astype(np.float32))   # [dh, s]
    sinT = np.ascontiguousarray(np.sin(emb).T.astype(np.float32))
    ssinT = sinT.copy()
    ssinT[: DH // 2] *= -1.0

    perm64 = np.zeros((128, 128), np.float32)
    for m in range(128):
        perm64[(m + 64) % 128, m] = 1.0

    wq = np.asarray(wq, np.float32)
    wk = np.asarray(wk, np.float32)
    wv = np.asarray(wv, np.float32)
    wo = np.asarray(wo, np.float32)

    in_maps = []
    for c in range(NCORES):
        cs = slice(DHC * c, DHC * (c + 1))
        wqT = np.ascontiguousarray(wq[cs, :].T).reshape(kb, 128, DHC)
        wkT = np.ascontiguousarray(wk[cs, :].T).reshape(kb, 128, DHC)
        wvT = np.ascontiguousarray(wv[cs, :].T).reshape(kb, 128, DHC)
        woT = np.ascontiguousarray(wo[:, cs].T).reshape(HPC, 128, d)
        in_maps.append(dict(
            xT=xT, wqT=wqT, wkT=wkT, wvT=wvT, woT=woT,
            cosT=cosT, ssinT=ssinT,
            ones_col=np.ones((128, 1), np.float32),
            ones_row=np.ones((1, 128), np.float32),
            perm64=perm64,
        ))
    return in_maps


def kernel(hidden_states, wq, wk, wv, wo, position_ids):
    from concourse.bass_utils import run_bass_kernel_spmd

    b, s, d = hidden_states.shape
    if "nc" not in _CACHE:
        _CACHE["nc"] = _build(s, d)
    nc = _CACHE["nc"]

    in_maps = _prepare_inputs(hidden_states, wq, wk, wv, wo, position_ids, s, d)
    res = run_bass_kernel_spmd(nc, in_maps, core_ids=list(range(NCORES)))
    y = np.zeros((s, d), np.float64)
    for c in range(NCORES):
        y += res.results[c]["y"].reshape(s, d).astype(np.float64)
    return y.astype(np.float32).reshape(b, s, d)
